# revision 41
# baseline (speedup 1.0000x reference)
"""Multi-head attention (B=1, S=4096, D=1024, H=16, causal) on 8 Trainium2
NeuronCores.

Sharding: tensor-parallel over heads — each core owns 2 heads (128 of the
1024 projection dims). Wq/Wk/Wv are split column-wise, Wo row-wise; each
core computes a full [S, D] partial of the output projection (bf16) and the
all-reduce is done on the host by summing the 8 partials (+ Wo_b once).

All matmul operands are bf16 (f32 PSUM accumulation): same 1 cycle/row PE
rate as f32r but FWL-eligible weight loads, half the DMA/SBUF traffic, and
2x DVE modes where applicable.

Per-core device kernel:
  qT/kT projections produce [c=128, S] bf16 (weight-stationary, contract
  D streams from host-pretransposed inputs); the two heads live on partition
  halves 0-63 / 64-127 so the per-head score matmuls (contract 64) auto-
  derive PE row tiles (0,0)/(64,0). Per pair, head 1's score matmuls are
  emitted FIRST: their psS buffer is freed by the later of the two exps of
  the previous pair, so all four matmuls release together and the two
  K=64 row-groups genuinely overlap in the array.
  v is projected directly into [s, c] layout (x-subtile stationary) and
  bias-added into augmented [ones|dims] slots per head: the attn@V matmul
  lands the softmax denominator replicated on PSUM partitions 0-63
  (numerators on 64-127) so normalization is reciprocal_approx_fast + one
  multiply, no broadcast needed.
  Scores are computed transposed (scoresT[k, q]) so softmax exp is the PSUM
  eviction (ACT, scale=1/8, bf16 out) into one [128, 2048] tile per pair;
  partial diagonal 128-bands are zeroed by a single two-head Pool-engine
  mask multiply; fully-masked blocks are skipped. Optionally every
  DVE_EXP_MOD-th unmasked pair computes head 1's exp on the Vector engine
  via Schraudolph exponent-stuffing to off-load the ACT exp stream.
  The normalized bf16 [c, q] tiles for both heads land in one [128, q] tile
  so the final Wo projection is a single K=128 matmul per output block.
  Projections and Wo blocks are emitted as filler units interleaved into
  the attention pair loop, keeping both the PE dense (HAM stays
  un-throttled) and the ACT exp stream gapless across s-tiles; attn@V lags
  its exp by one pair so the in-order PE queue never stalls.
"""

import numpy as np
import ml_dtypes

D = 1024
H = 16
DK = D // H  # 64
S = 4096
NCORES = 8
CD = 128          # c-dims (2 heads) per core
ST = 512          # s/q tile
NST = S // ST     # 8
KB = 128          # k block
NKB = S // KB     # 32
SLOT = 128        # v_sb cols per head per k-block (64 ones + 64 dims)

_compiled = [None]

# Softmax-exp load balancing: ScalarE (ACT) is the only engine with a real
# exp, and its ~109us of exp streaming is a co-bottleneck with the PE. For
# every DVE_EXP_MOD-th fully-unmasked pair, head 1's exp is computed on the
# Vector engine instead via Schraudolph exponent-stuffing: writing
# int16(score * 16*log2(e) + (16256 - 128*sigma)) and reinterpreting the
# bits as bfloat16 yields 2^(score*log2(e)/8) = exp(score/8) to ~3-4% per
# element; the softmax ratio cancels most of it. 0 disables.
DVE_EXP_MOD = 3
EXP_K1 = 16 * 1.4426950408889634
EXP_K2 = 16256.0 - 128 * 0.0434609


def _build():
    import concourse.bacc as bacc
    import concourse.mybir as mybir
    import concourse.tile as tile

    f32 = mybir.dt.float32
    f32r = mybir.dt.float32r
    bf16 = mybir.dt.bfloat16
    EXP = mybir.ActivationFunctionType.Exp
    MULT = mybir.AluOpType.mult
    ADD = mybir.AluOpType.add

    nc = bacc.Bacc(None, target_bir_lowering=False)

    XQ = nc.dram_tensor("xq", [128, 8, S], bf16, kind="ExternalInput")
    XK = nc.dram_tensor("xk", [128, 8, S], bf16, kind="ExternalInput")
    XV = nc.dram_tensor("xv", [128, 8, S], bf16, kind="ExternalInput")
    WQ = nc.dram_tensor("wq", [128, 8, CD], bf16, kind="ExternalInput")
    WK = nc.dram_tensor("wk", [128, 8, CD], bf16, kind="ExternalInput")
    WV = nc.dram_tensor("wv", [128, 8, CD], bf16, kind="ExternalInput")
    BQ = nc.dram_tensor("bq", [CD, 1], f32, kind="ExternalInput")
    BK = nc.dram_tensor("bk", [CD, 1], f32, kind="ExternalInput")
    BVB = nc.dram_tensor("bvb", [128, 512], bf16, kind="ExternalInput")
    WOR = nc.dram_tensor("wor", [CD, D], bf16, kind="ExternalInput")
    MSK = nc.dram_tensor("msk", [KB, 2 * KB], bf16, kind="ExternalInput")
    OUT = nc.dram_tensor("out", [S, D], bf16, kind="ExternalOutput")

    with tile.TileContext(nc) as tc:
        with (
            tc.tile_pool(name="const", bufs=1) as const,
            tc.tile_pool(name="qin", bufs=3) as qin_p,
            tc.tile_pool(name="kin", bufs=3) as kin_p,
            tc.tile_pool(name="vin", bufs=3) as vin_p,
            tc.tile_pool(name="expp", bufs=6) as exp_p,
            tc.tile_pool(name="rsb", bufs=4) as rsb_p,
            tc.tile_pool(name="wlp", bufs=3) as wl_p,
            tc.tile_pool(name="oout", bufs=6) as oout_p,
            tc.tile_pool(name="psA", bufs=2, space="PSUM") as psA,
            tc.tile_pool(name="psS", bufs=2, space="PSUM") as psS,
            tc.tile_pool(name="psO", bufs=2, space="PSUM") as psO,
        ):
            # ---- static SBUF tensors ----
            qT_sb = const.tile([CD, S], bf16, tag="qTl")
            kT_sb = const.tile([CD, S], bf16, tag="kT")
            v_sb = const.tile([128, NKB, 2 * SLOT], bf16, tag="vsb")

            wq_sb = const.tile([128, 8, CD], bf16, tag="wq")
            wk_sb = const.tile([128, 8, CD], bf16, tag="wk")
            wv_sb = const.tile([128, 8, CD], bf16, tag="wv")
            woR = const.tile([CD, D], bf16, tag="woR")
            mask_sb = const.tile([KB, 2 * KB], bf16, tag="mask")
            bq_sb = const.tile([CD, 1], f32, tag="bq")
            bk_sb = const.tile([CD, 1], f32, tag="bk")
            bvb_sb = const.tile([128, 512], bf16, tag="bvb")

            woL_tiles = {}
            prefetched = {}
            full_cnt = [0]

            def fetch(st, src, in_pool, name, split=1):
                xin = in_pool.tile([128, 8, ST], bf16, tag="xin",
                                   name=f"xin_{name}{st}")
                g = 8 // split
                for i in range(split):
                    nc.sync.dma_start(
                        out=xin[:, i * g : (i + 1) * g, :],
                        in_=src[:, i * g : (i + 1) * g,
                                st * ST : (st + 1) * ST],
                    )
                return xin

            # critical consts first, split so the t=0..3 projection matmuls
            # can start before the second halves land
            nc.sync.dma_start(out=wq_sb[:, 0:4, :], in_=WQ[:, 0:4, :])
            prefetched[("q", 0)] = fetch(0, XQ, qin_p, "q", split=2)
            nc.sync.dma_start(out=wq_sb[:, 4:8, :], in_=WQ[:, 4:8, :])
            nc.sync.dma_start(out=bq_sb[:], in_=BQ[:])
            nc.sync.dma_start(out=wk_sb[:, 0:4, :], in_=WK[:, 0:4, :])
            prefetched[("k", 0)] = fetch(0, XK, kin_p, "k", split=2)
            nc.sync.dma_start(out=wk_sb[:, 4:8, :], in_=WK[:, 4:8, :])
            nc.sync.dma_start(out=bk_sb[:], in_=BK[:])
            # mask before wv: the first diagonal tri-matmul needs it early
            nc.sync.dma_start(out=mask_sb[:], in_=MSK[:])
            nc.sync.dma_start(out=wv_sb[:], in_=WV[:])
            prefetched[("v", 0)] = fetch(0, XV, vin_p, "v")
            # tile-1 inputs next: attn(0) is short, so proj(1) starts early;
            # bvb/woR are not needed until late in attn(0)/attn(1)
            prefetched[("q", 1)] = fetch(1, XQ, qin_p, "q")
            prefetched[("k", 1)] = fetch(1, XK, kin_p, "k")
            nc.sync.dma_start(out=bvb_sb[:], in_=BVB[:])
            prefetched[("v", 1)] = fetch(1, XV, vin_p, "v")
            nc.sync.dma_start(out=woR[:], in_=WOR[:])

            # HAM warm-up: ~20 dummy matmuls off the just-landed wq chunk
            # keep the PE busy through its 3.4us activity window while the
            # first x tiles are still in flight, so the real projection
            # stream starts at 2.4GHz instead of 1.2GHz
            wu = psA.tile([128, ST], f32, tag="pp", name="warmup")
            for i in range(20):
                nc.tensor.matmul(
                    wu[:, 0:KB],
                    lhsT=wq_sb[:, i % 4, :],
                    rhs=wq_sb[:, (i + 1) % 4, :],
                    start=True, stop=True,
                )

            # ones blocks of the augmented v slots (cols 0-63 per head
            # slot => attn@V lands denominators on PSUM partitions 0-63,
            # numerators on 64-127)
            nc.gpsimd.memset(v_sb[:, :, 0:DK], 1.0)
            nc.gpsimd.memset(v_sb[:, :, SLOT : SLOT + DK], 1.0)

            def get_in(st, name, src, in_pool):
                xin = prefetched.pop((name, st), None)
                if xin is None:
                    xin = fetch(st, src, in_pool, name)
                return xin

            def proj_units(st, xq, xk, xv):
                """Projection of s-tile st as schedulable PE work units."""
                state = {}

                def qk_part(xin, w_sb, b_sb, dst_ap, key, lo, hi):
                    def run():
                        if key not in state:
                            state[key] = psA.tile([128, ST], f32, tag="pp",
                                                  name=f"pp{key}{st}")
                        ps = state[key]
                        for t in range(lo, hi):
                            nc.tensor.matmul(
                                ps[:],
                                lhsT=w_sb[:, t, :],
                                rhs=xin[:, t, :],
                                start=(t == 0),
                                stop=(t == 7),
                            )
                        if hi == 8:
                            nc.vector.tensor_scalar_add(dst_ap, ps[:],
                                                        b_sb[:])
                    return run

                def v_part(qb):
                    def run():
                        if "v" not in state:
                            state["v"] = psA.tile([128, 4, 128], f32,
                                                  tag="pp", name=f"pv{st}")
                        pv = state["v"]
                        for t in range(8):
                            nc.tensor.matmul(
                                pv[:, qb, :],
                                lhsT=xv[:, t, qb * 128 : (qb + 1) * 128],
                                rhs=wv_sb[:, t, :],
                                start=(t == 0),
                                stop=(t == 7),
                            )
                        if qb == 3:
                            # bias-add + pack into augmented slots (skip the
                            # ones columns); DVE: GPSIMD cannot read PSUM
                            nc.vector.tensor_tensor(
                                out=v4[:, 4 * st : 4 * st + 4, :, DK:SLOT],
                                in0=pv.rearrange("p k (h c) -> p k h c", h=2),
                                in1=bvb4[:],
                                op=ADD,
                            )
                    return run

                qdst = qT_sb[:, st * ST : (st + 1) * ST]
                kdst = kT_sb[:, st * ST : (st + 1) * ST]
                return [
                    qk_part(xq, wq_sb, bq_sb, qdst, "q", 0, 4),
                    qk_part(xq, wq_sb, bq_sb, qdst, "q", 4, 8),
                    qk_part(xk, wk_sb, bk_sb, kdst, "k", 0, 4),
                    qk_part(xk, wk_sb, bk_sb, kdst, "k", 4, 8),
                    v_part(0), v_part(1), v_part(2), v_part(3),
                ]

            v4 = v_sb.rearrange("p n (h c) -> p n h c", h=2)
            bvb4 = bvb_sb.rearrange("p (k h c) -> p k h c", k=4, h=2)

            def attn(qt, filler):
                npr = 2 * qt + 2
                po = {}
                for h in (0, 1):
                    po[h] = psO.tile([128, ST], f32, tag="po",
                                     name=f"po{qt}_{h}")

                def attnv(pr, ex):
                    # attn @ V (+ones cols => denominators on po
                    # partitions 0-63, numerators on 64-127)
                    for h in (0, 1):
                        for j in range(2):
                            kb = 2 * pr + j
                            rel = kb - 4 * qt
                            c0 = 128 * rel if rel > 0 else 0
                            nc.tensor.matmul(
                                po[h][:, c0:ST],
                                lhsT=v_sb[:, kb, h * KB : (h + 1) * KB],
                                rhs=ex[:, h, j * ST + c0 : (j + 1) * ST],
                                start=(pr == 0 and j == 0),
                                stop=(pr == npr - 1 and j == 1),
                            )

                prev = None  # (pr, ex) whose attn@V is still pending
                for pr in range(npr):
                    rels = [2 * pr + j - 4 * qt for j in (0, 1)]
                    ps = {}
                    for h in (0, 1):
                        ps[h] = psS.tile([128, 2 * ST], f32, tag="ps",
                                         name=f"ps{qt}_{h}_{pr}")
                    # scores: h1 emitted first — its psS buffer is freed by
                    # the LATER of the two exps of pr-1, so by the time the
                    # h1 matmul's wait clears, h0's has cleared too and the
                    # two K=64 matmuls issue back-to-back into different PE
                    # row-groups (concurrent in the array). Diagonal blocks
                    # get the causal mask accumulated right into PSUM as an
                    # identity-weighted -240 upper-triangle (exp -> ~1e-13),
                    # keeping GpSimd out of the exp->attn@V chain
                    for j in range(2):
                        kb = 2 * pr + j
                        rel = rels[j]
                        c0 = 128 * rel if rel > 0 else 0
                        diag = rel >= 0
                        for h in (1, 0):
                            nc.tensor.matmul(
                                ps[h][:, j * ST + c0 : (j + 1) * ST],
                                lhsT=kT_sb[64 * h : 64 * h + 64,
                                           kb * KB : (kb + 1) * KB],
                                rhs=qT_sb[64 * h : 64 * h + 64,
                                          qt * ST + c0 : (qt + 1) * ST],
                                start=True,
                                stop=not diag,
                            )
                        if diag:
                            b0 = j * ST + 128 * rel
                            for h in (1, 0):
                                nc.tensor.matmul(
                                    ps[h][:, b0 : b0 + 128],
                                    lhsT=mask_sb[:, 0:KB],
                                    rhs=mask_sb[:, KB : 2 * KB],
                                    start=False,
                                    stop=True,
                                )
                    # exp: both heads into ONE [128, 2048] sbuf tile so the
                    # diagonal mask-multiplies cover both heads in one op
                    ext = exp_p.tile([128, 2, 2 * ST], bf16, tag="ex",
                                     name=f"ex{qt}_{pr}")
                    dve_h1 = False
                    if rels[1] < 0 and DVE_EXP_MOD:
                        full_cnt[0] += 1
                        dve_h1 = full_cnt[0] % DVE_EXP_MOD == 0
                    for h in (0, 1):
                        if rels[1] >= 0:  # diagonal pair: narrow exps that
                            # skip the never-read below-diagonal region
                            for j in range(2):
                                c0 = 128 * max(rels[j], 0)
                                nc.scalar.activation(
                                    ext[:, h, j * ST + c0 : (j + 1) * ST],
                                    ps[h][:, j * ST + c0 : (j + 1) * ST],
                                    EXP, scale=0.125,
                                )
                        elif h == 1 and dve_h1:
                            nc.vector.tensor_scalar(
                                ext[:, h, :].bitcast(mybir.dt.int16),
                                ps[h][:], EXP_K1, EXP_K2, MULT, ADD,
                            )
                        else:
                            nc.scalar.activation(ext[:, h, :], ps[h][:],
                                                 EXP, scale=0.125)
                    # attn@V lags one pair so PE never stalls on this exp
                    if prev is not None:
                        attnv(*prev)
                    prev = (pr, ext)
                    # interleave pending proj/Wo units, spread evenly
                    filler(-(npr - pr))
                attnv(*prev)
                if qt == NST - 1:
                    # tail: the serial normalize chain leaves the PE idle
                    # longer than the HAM window — keep it warm with dummy
                    # matmuls so the Wo tail runs at 2.4GHz, not 1.2
                    wu2 = psA.tile([128, ST], f32, tag="pp", name="warm2")
                    for i in range(14):
                        nc.tensor.matmul(
                            wu2[:, 0:KB],
                            lhsT=wq_sb[:, i % 4, :],
                            rhs=wq_sb[:, (i + 1) % 4, :],
                            start=True, stop=True,
                        )
                # normalize: woL[h*64:(h+1)*64, :] = po[h][64:128] / denom
                # (note: reciprocal_approx_fast requires base partition 0 —
                # nonzero-base operands return garbage on HW, probed)
                woL = wl_p.tile([128, ST], bf16, tag="wl", name=f"wl{qt}")
                for h in (0, 1):
                    r_sb = rsb_p.tile([DK, ST], f32, tag="r",
                                      name=f"r{qt}_{h}")
                    nc.vector.reciprocal_approx_fast(out=r_sb[:],
                                                     in_=po[h][0:64, :])
                    nc.vector.tensor_tensor(
                        out=woL[64 * h : 64 * h + 64, :],
                        in0=po[h][64:128, :], in1=r_sb[:], op=MULT,
                    )
                woL_tiles[qt] = woL

            def wo_units(qt):
                """8 closures, each one output block of the Wo projection.
                The two 512-col halves of a 128-row block share one sbuf
                tile and go to HBM as a single 1024-wide DMA."""
                wl = woL_tiles.pop(qt)
                obs = {}

                def unit(qb, nt):
                    def run():
                        q0 = qt * ST + qb * 128
                        pw = psA.tile([128, ST], f32, tag="pp",
                                      name=f"pw{qt}_{qb}_{nt}")
                        nc.tensor.matmul(
                            pw[:],
                            lhsT=wl[:, qb * 128 : (qb + 1) * 128],
                            rhs=woR[:, nt * ST : (nt + 1) * ST],
                            start=True, stop=True,
                        )
                        if nt == 0:
                            obs[qb] = oout_p.tile([128, 2 * ST], bf16,
                                                  tag="ob",
                                                  name=f"ob{qt}_{qb}")
                        ob = obs[qb]
                        if qt == NST - 1 and (qb + nt) % 2:
                            # tail: ACT is done with exps — share the
                            # PSUM eviction between Scalar and Vector
                            nc.scalar.copy(ob[:, nt * ST : (nt + 1) * ST],
                                           pw[:])
                        else:
                            nc.vector.tensor_copy(
                                ob[:, nt * ST : (nt + 1) * ST], pw[:])
                        if qt == NST - 1:
                            # tail: store each half as soon as it lands so
                            # the final DMA overlaps the remaining evicts
                            nc.sync.dma_start(
                                out=OUT[q0 : q0 + 128,
                                        nt * ST : (nt + 1) * ST],
                                in_=ob[:, nt * ST : (nt + 1) * ST],
                            )
                        elif nt == 1:
                            nc.sync.dma_start(
                                out=OUT[q0 : q0 + 128, :], in_=ob[:],
                            )
                    return run

                return [unit(qb, nt) for qb in range(4) for nt in range(2)]

            # two filler queues: projections gate the next tile's scores
            # (must drain at the tile boundary); Wo blocks are deferrable
            # and carry across boundaries so the boundary drain never
            # starves the ACT exp stream
            pend_proj = []
            pend_wo = []

            reserve = [4]

            def navail():
                return len(pend_proj) + len(pend_wo)

            def filler(n):
                # n < 0: spread -> emit ceil((len-R)/|n|), holding ~R units
                # back as PE filler for the next tile's ACT-bound start;
                # n > 0: emit up to n (force-drain)
                if n < 0:
                    n = -((navail() - reserve[0]) // n)
                for _ in range(min(n, navail())):
                    (pend_proj if pend_proj else pend_wo).pop(0)()

            for st in range(NST):
                if st == 0:
                    # q/k projections run inline (the first scores need
                    # them); the v projection overlaps the first exps as
                    # filler units
                    xq = get_in(0, "q", XQ, qin_p)
                    xk = get_in(0, "k", XK, kin_p)
                    xv = get_in(0, "v", XV, vin_p)
                    u0 = proj_units(0, xq, xk, xv)
                    for u in u0[:4]:
                        u()
                    pend_proj.extend(u0[4:])
                else:
                    # leftover proj units for this tile must precede its
                    # scores; deferred wo units carry over
                    while pend_proj:
                        pend_proj.pop(0)()
                if st + 1 < NST:
                    xq = get_in(st + 1, "q", XQ, qin_p)
                    xk = get_in(st + 1, "k", XK, kin_p)
                    xv = get_in(st + 1, "v", XV, vin_p)
                    pend_proj.extend(proj_units(st + 1, xq, xk, xv))
                if st >= 1:
                    # wo for qt=st-1: drained by filler inside attn(st)
                    pend_wo.extend(wo_units(st - 1))
                reserve[0] = 0 if st in (0, NST - 1) else 4
                attn(st, filler)
            # drain the tail
            filler(navail())
            pend_wo.extend(wo_units(NST - 1))
            filler(navail())

    nc.compile()
    return nc


def _prep_inputs(Q, K, V, Wq_w, Wq_b, Wk_w, Wk_b, Wv_w, Wv_b, Wo_w, Wo_b):
    bf = ml_dtypes.bfloat16
    f = np.float32

    def xprep(X):
        # [S, D] -> [128, 8, S]: x[p, t, s] = X[s, t*128+p]
        return np.ascontiguousarray(
            X[0].T.reshape(8, 128, S).transpose(1, 0, 2).astype(bf)
        )

    def wprep(Wslice):
        # Wslice [CD, D] -> [128, 8, CD]: w[p, t, c] = Wslice[c, t*128+p]
        return np.ascontiguousarray(
            Wslice.T.reshape(8, 128, CD).transpose(1, 0, 2).astype(bf)
        )

    XQp, XKp, XVp = xprep(Q), xprep(K), xprep(V)
    p = np.arange(KB)[:, None]
    fidx = np.arange(KB)[None, :]
    ident = np.eye(KB, dtype=np.float32)
    tri = np.where(p > fidx, -240.0, 0.0)
    msk = np.ascontiguousarray(np.concatenate([ident, tri], 1)).astype(bf)
    WoT = np.ascontiguousarray(Wo_w.T, dtype=f)  # [in, out]

    in_maps = []
    for c in range(NCORES):
        c0 = CD * c
        in_maps.append({
            "xq": XQp, "xk": XKp, "xv": XVp,
            "wq": wprep(Wq_w[c0 : c0 + CD, :]),
            "wk": wprep(Wk_w[c0 : c0 + CD, :]),
            "wv": wprep(Wv_w[c0 : c0 + CD, :]),
            "bq": np.ascontiguousarray(Wq_b[c0 : c0 + CD, None], dtype=f),
            "bk": np.ascontiguousarray(Wk_b[c0 : c0 + CD, None], dtype=f),
            "bvb": np.ascontiguousarray(
                np.broadcast_to(np.tile(Wv_b[c0 : c0 + CD], 4), (128, 512))
            ).astype(bf),
            "wor": np.ascontiguousarray(WoT[c0 : c0 + CD, :], dtype=bf),
            "msk": msk,
        })
    return in_maps


def _numpy_fallback(Q, K, V, Wq_w, Wq_b, Wk_w, Wk_b, Wv_w, Wv_b, Wo_w, Wo_b,
                    mask):
    q = (Q @ Wq_w.T + Wq_b).reshape(1, S, H, DK).transpose(0, 2, 1, 3)
    k = (K @ Wk_w.T + Wk_b).reshape(1, S, H, DK).transpose(0, 2, 1, 3)
    v = (V @ Wv_w.T + Wv_b).reshape(1, S, H, DK).transpose(0, 2, 1, 3)
    scores = np.einsum("bhqd,bhkd->bhqk", q, k) / np.sqrt(DK).astype(np.float32)
    scores = np.where(mask == 0, np.float32(-1e9), scores)
    scores -= scores.max(axis=-1, keepdims=True)
    e = np.exp(scores)
    attn = e / e.sum(axis=-1, keepdims=True)
    out = np.einsum("bhqk,bhkd->bhqd", attn, v)
    out = out.transpose(0, 2, 1, 3).reshape(1, S, D)
    return (out @ Wo_w.T + Wo_b).astype(np.float32)


def kernel(Q, K, V, Wq_w, Wq_b, Wk_w, Wk_b, Wv_w, Wv_b, Wo_w, Wo_b, mask,
           **run_kwargs):
    Q = np.asarray(Q); K = np.asarray(K); V = np.asarray(V)
    Wq_w = np.asarray(Wq_w); Wq_b = np.asarray(Wq_b)
    Wk_w = np.asarray(Wk_w); Wk_b = np.asarray(Wk_b)
    Wv_w = np.asarray(Wv_w); Wv_b = np.asarray(Wv_b)
    Wo_w = np.asarray(Wo_w); Wo_b = np.asarray(Wo_b)
    mask = np.asarray(mask)

    causal = np.array_equal(
        mask.reshape(S, S), np.tril(np.ones((S, S), mask.dtype))
    )
    if not causal:
        return _numpy_fallback(Q, K, V, Wq_w, Wq_b, Wk_w, Wk_b, Wv_w, Wv_b,
                               Wo_w, Wo_b, mask)

    from concourse.bass_utils import run_bass_kernel_spmd

    if _compiled[0] is None:
        _compiled[0] = _build()
    nc = _compiled[0]

    in_maps = _prep_inputs(Q, K, V, Wq_w, Wq_b, Wk_w, Wk_b, Wv_w, Wv_b,
                           Wo_w, Wo_b)
    res = run_bass_kernel_spmd(nc, in_maps, list(range(NCORES)), **run_kwargs)
    out = np.zeros((S, D), np.float32)
    for cres in res.results:
        out += np.asarray(cres["out"], dtype=np.float32)
    out += Wo_b.astype(np.float32)
    if run_kwargs:
        kernel.last_result = res
    return out.reshape(1, S, D).astype(np.float32)



# revision 42
# speedup vs baseline: 1.0282x; 1.0282x over previous
"""Multi-head attention (B=1, S=4096, D=1024, H=16, causal) on 8 Trainium2
NeuronCores.

Sharding: tensor-parallel over heads — each core owns 2 heads (128 of the
1024 projection dims). Wq/Wk/Wv are split column-wise, Wo row-wise; each
core computes a full [S, D] partial of the output projection (bf16) and the
all-reduce is done on the host by summing the 8 partials (+ Wo_b once).

All matmul operands are bf16 (f32 PSUM accumulation): same 1 cycle/row PE
rate as f32r but FWL-eligible weight loads, half the DMA/SBUF traffic, and
2x DVE modes where applicable.

Per-core device kernel:
  qT/kT projections produce [c=128, S] bf16 (weight-stationary, contract
  D streams from host-pretransposed inputs); the two heads live on partition
  halves 0-63 / 64-127 so the per-head score matmuls (contract 64) auto-
  derive PE row tiles (0,0)/(64,0). Per pair, head 1's score matmuls are
  emitted FIRST: their psS buffer is freed by the later of the two exps of
  the previous pair, so all four matmuls release together and the two
  K=64 row-groups genuinely overlap in the array.
  v is projected directly into [s, c] layout (x-subtile stationary) and
  bias-added into augmented [ones|dims] slots per head: the attn@V matmul
  lands the softmax denominator replicated on PSUM partitions 0-63
  (numerators on 64-127) so normalization is reciprocal_approx_fast + one
  multiply, no broadcast needed.
  Scores are computed transposed (scoresT[k, q]) so softmax exp is the PSUM
  eviction (ACT, scale=1/8, bf16 out) into one [128, 2048] tile per pair;
  partial diagonal 128-bands are zeroed by a single two-head Pool-engine
  mask multiply; fully-masked blocks are skipped. Optionally every
  DVE_EXP_MOD-th unmasked pair computes head 1's exp on the Vector engine
  via Schraudolph exponent-stuffing to off-load the ACT exp stream.
  The normalized bf16 [c, q] tiles for both heads land in one [128, q] tile
  so the final Wo projection is a single K=128 matmul per output block.
  Projections and Wo blocks are emitted as filler units interleaved into
  the attention pair loop, keeping both the PE dense (HAM stays
  un-throttled) and the ACT exp stream gapless across s-tiles; attn@V lags
  its exp by one pair so the in-order PE queue never stalls.
"""

import numpy as np
import ml_dtypes

D = 1024
H = 16
DK = D // H  # 64
S = 4096
NCORES = 8
CD = 128          # c-dims (2 heads) per core
ST = 512          # s/q tile
NST = S // ST     # 8
KB = 128          # k block
NKB = S // KB     # 32
SLOT = 128        # v_sb cols per head per k-block (64 ones + 64 dims)

_compiled = [None]

# Softmax-exp load balancing: ScalarE (ACT) is the only engine with a real
# exp, and its ~109us of exp streaming is a co-bottleneck with the PE. For
# every DVE_EXP_MOD-th fully-unmasked pair, head 1's exp is computed on the
# Vector engine instead via Schraudolph exponent-stuffing: writing
# int16(score * 16*log2(e) + (16256 - 128*sigma)) and reinterpreting the
# bits as bfloat16 yields 2^(score*log2(e)/8) = exp(score/8) to ~3-4% per
# element; the softmax ratio cancels most of it. 0 disables.
DVE_EXP_MOD = 0
EXP_K1 = 16 * 1.4426950408889634
EXP_K2 = 16256.0 - 128 * 0.0434609


def _build():
    import concourse.bacc as bacc
    import concourse.mybir as mybir
    import concourse.tile as tile

    f32 = mybir.dt.float32
    f32r = mybir.dt.float32r
    bf16 = mybir.dt.bfloat16
    EXP = mybir.ActivationFunctionType.Exp
    MULT = mybir.AluOpType.mult
    ADD = mybir.AluOpType.add

    nc = bacc.Bacc(None, target_bir_lowering=False)

    XQ = nc.dram_tensor("xq", [128, 8, S], bf16, kind="ExternalInput")
    XK = nc.dram_tensor("xk", [128, 8, S], bf16, kind="ExternalInput")
    XV = nc.dram_tensor("xv", [128, 8, S], bf16, kind="ExternalInput")
    WQ = nc.dram_tensor("wq", [128, 8, CD], bf16, kind="ExternalInput")
    WK = nc.dram_tensor("wk", [128, 8, CD], bf16, kind="ExternalInput")
    WV = nc.dram_tensor("wv", [128, 8, CD], bf16, kind="ExternalInput")
    BQ = nc.dram_tensor("bq", [CD, 1], f32, kind="ExternalInput")
    BK = nc.dram_tensor("bk", [CD, 1], f32, kind="ExternalInput")
    BVB = nc.dram_tensor("bvb", [128, 512], bf16, kind="ExternalInput")
    WOR = nc.dram_tensor("wor", [CD, D], bf16, kind="ExternalInput")
    MSK = nc.dram_tensor("msk", [KB, 2 * KB], bf16, kind="ExternalInput")
    OUT = nc.dram_tensor("out", [S, D], bf16, kind="ExternalOutput")

    with tile.TileContext(nc) as tc:
        with (
            tc.tile_pool(name="const", bufs=1) as const,
            tc.tile_pool(name="qin", bufs=3) as qin_p,
            tc.tile_pool(name="kin", bufs=3) as kin_p,
            tc.tile_pool(name="vin", bufs=3) as vin_p,
            tc.tile_pool(name="expp", bufs=6) as exp_p,
            tc.tile_pool(name="rsb", bufs=4) as rsb_p,
            tc.tile_pool(name="wlp", bufs=3) as wl_p,
            tc.tile_pool(name="oout", bufs=6) as oout_p,
            tc.tile_pool(name="psA", bufs=2, space="PSUM") as psA,
            tc.tile_pool(name="psS", bufs=2, space="PSUM") as psS,
            tc.tile_pool(name="psO", bufs=2, space="PSUM") as psO,
        ):
            # ---- static SBUF tensors ----
            qT_sb = const.tile([CD, S], bf16, tag="qTl")
            kT_sb = const.tile([CD, S], bf16, tag="kT")
            v_sb = const.tile([128, NKB, 2 * SLOT], bf16, tag="vsb")

            wq_sb = const.tile([128, 8, CD], bf16, tag="wq")
            wk_sb = const.tile([128, 8, CD], bf16, tag="wk")
            wv_sb = const.tile([128, 8, CD], bf16, tag="wv")
            woR = const.tile([CD, D], bf16, tag="woR")
            mask_sb = const.tile([KB, 2 * KB], bf16, tag="mask")
            bq_sb = const.tile([CD, 1], f32, tag="bq")
            bk_sb = const.tile([CD, 1], f32, tag="bk")
            bvb_sb = const.tile([128, 512], bf16, tag="bvb")

            woL_tiles = {}
            prefetched = {}
            full_cnt = [0]

            def fetch(st, src, in_pool, name, split=1):
                xin = in_pool.tile([128, 8, ST], bf16, tag="xin",
                                   name=f"xin_{name}{st}")
                g = 8 // split
                for i in range(split):
                    nc.sync.dma_start(
                        out=xin[:, i * g : (i + 1) * g, :],
                        in_=src[:, i * g : (i + 1) * g,
                                st * ST : (st + 1) * ST],
                    )
                return xin

            # critical consts first, split so the t=0..3 projection matmuls
            # can start before the second halves land
            nc.sync.dma_start(out=wq_sb[:, 0:4, :], in_=WQ[:, 0:4, :])
            prefetched[("q", 0)] = fetch(0, XQ, qin_p, "q", split=2)
            nc.sync.dma_start(out=wq_sb[:, 4:8, :], in_=WQ[:, 4:8, :])
            nc.sync.dma_start(out=bq_sb[:], in_=BQ[:])
            nc.sync.dma_start(out=wk_sb[:, 0:4, :], in_=WK[:, 0:4, :])
            prefetched[("k", 0)] = fetch(0, XK, kin_p, "k", split=2)
            nc.sync.dma_start(out=wk_sb[:, 4:8, :], in_=WK[:, 4:8, :])
            nc.sync.dma_start(out=bk_sb[:], in_=BK[:])
            # mask before wv: the first diagonal tri-matmul needs it early
            nc.sync.dma_start(out=mask_sb[:], in_=MSK[:])
            nc.sync.dma_start(out=wv_sb[:], in_=WV[:])
            prefetched[("v", 0)] = fetch(0, XV, vin_p, "v")
            # tile-1 inputs next: attn(0) is short, so proj(1) starts early;
            # bvb/woR are not needed until late in attn(0)/attn(1)
            prefetched[("q", 1)] = fetch(1, XQ, qin_p, "q")
            prefetched[("k", 1)] = fetch(1, XK, kin_p, "k")
            nc.sync.dma_start(out=bvb_sb[:], in_=BVB[:])
            prefetched[("v", 1)] = fetch(1, XV, vin_p, "v")
            nc.sync.dma_start(out=woR[:], in_=WOR[:])

            # HAM warm-up: ~20 dummy matmuls off the just-landed wq chunk
            # keep the PE busy through its 3.4us activity window while the
            # first x tiles are still in flight, so the real projection
            # stream starts at 2.4GHz instead of 1.2GHz
            wu = psA.tile([128, ST], f32, tag="pp", name="warmup")
            for i in range(20):
                nc.tensor.matmul(
                    wu[:, 0:KB],
                    lhsT=wq_sb[:, i % 4, :],
                    rhs=wq_sb[:, (i + 1) % 4, :],
                    start=True, stop=True,
                )

            # ones blocks of the augmented v slots (cols 0-63 per head
            # slot => attn@V lands denominators on PSUM partitions 0-63,
            # numerators on 64-127)
            nc.gpsimd.memset(v_sb[:, :, 0:DK], 1.0)
            nc.gpsimd.memset(v_sb[:, :, SLOT : SLOT + DK], 1.0)

            def get_in(st, name, src, in_pool):
                xin = prefetched.pop((name, st), None)
                if xin is None:
                    xin = fetch(st, src, in_pool, name)
                return xin

            def proj_units(st, xq, xk, xv):
                """Projection of s-tile st as schedulable PE work units."""
                state = {}

                def qk_part(xin, w_sb, b_sb, dst_ap, key, lo, hi):
                    def run():
                        if key not in state:
                            state[key] = psA.tile([128, ST], f32, tag="pp",
                                                  name=f"pp{key}{st}")
                        ps = state[key]
                        for t in range(lo, hi):
                            nc.tensor.matmul(
                                ps[:],
                                lhsT=w_sb[:, t, :],
                                rhs=xin[:, t, :],
                                start=(t == 0),
                                stop=(t == 7),
                            )
                        if hi == 8:
                            nc.vector.tensor_scalar_add(dst_ap, ps[:],
                                                        b_sb[:])
                    return run

                def v_part(qb):
                    def run():
                        if "v" not in state:
                            state["v"] = psA.tile([128, 4, 128], f32,
                                                  tag="pp", name=f"pv{st}")
                        pv = state["v"]
                        for t in range(8):
                            nc.tensor.matmul(
                                pv[:, qb, :],
                                lhsT=xv[:, t, qb * 128 : (qb + 1) * 128],
                                rhs=wv_sb[:, t, :],
                                start=(t == 0),
                                stop=(t == 7),
                            )
                        if qb == 3:
                            # bias-add + pack into augmented slots (skip the
                            # ones columns); DVE: GPSIMD cannot read PSUM
                            nc.vector.tensor_tensor(
                                out=v4[:, 4 * st : 4 * st + 4, :, DK:SLOT],
                                in0=pv.rearrange("p k (h c) -> p k h c", h=2),
                                in1=bvb4[:],
                                op=ADD,
                            )
                    return run

                qdst = qT_sb[:, st * ST : (st + 1) * ST]
                kdst = kT_sb[:, st * ST : (st + 1) * ST]
                return [
                    qk_part(xq, wq_sb, bq_sb, qdst, "q", 0, 4),
                    qk_part(xq, wq_sb, bq_sb, qdst, "q", 4, 8),
                    qk_part(xk, wk_sb, bk_sb, kdst, "k", 0, 4),
                    qk_part(xk, wk_sb, bk_sb, kdst, "k", 4, 8),
                    v_part(0), v_part(1), v_part(2), v_part(3),
                ]

            v4 = v_sb.rearrange("p n (h c) -> p n h c", h=2)
            bvb4 = bvb_sb.rearrange("p (k h c) -> p k h c", k=4, h=2)

            def attn(qt, filler):
                npr = 2 * qt + 2
                po = {}
                for h in (0, 1):
                    po[h] = psO.tile([128, ST], f32, tag="po",
                                     name=f"po{qt}_{h}")

                def attnv(pr, ex):
                    # attn @ V (+ones cols => denominators on po
                    # partitions 0-63, numerators on 64-127)
                    for h in (0, 1):
                        for j in range(2):
                            kb = 2 * pr + j
                            rel = kb - 4 * qt
                            c0 = 128 * rel if rel > 0 else 0
                            nc.tensor.matmul(
                                po[h][:, c0:ST],
                                lhsT=v_sb[:, kb, h * KB : (h + 1) * KB],
                                rhs=ex[:, h, j * ST + c0 : (j + 1) * ST],
                                start=(pr == 0 and j == 0),
                                stop=(pr == npr - 1 and j == 1),
                            )

                prev = None  # (pr, ex) whose attn@V is still pending
                for pr in range(npr):
                    rels = [2 * pr + j - 4 * qt for j in (0, 1)]
                    ps = {}
                    for h in (0, 1):
                        ps[h] = psS.tile([128, 2 * ST], f32, tag="ps",
                                         name=f"ps{qt}_{h}_{pr}")
                    # scores: h1 emitted first — its psS buffer is freed by
                    # the LATER of the two exps of pr-1, so by the time the
                    # h1 matmul's wait clears, h0's has cleared too and the
                    # two K=64 matmuls issue back-to-back into different PE
                    # row-groups (concurrent in the array). Diagonal blocks
                    # get the causal mask accumulated right into PSUM as an
                    # identity-weighted -240 upper-triangle (exp -> ~1e-13),
                    # keeping GpSimd out of the exp->attn@V chain
                    for j in range(2):
                        kb = 2 * pr + j
                        rel = rels[j]
                        c0 = 128 * rel if rel > 0 else 0
                        diag = rel >= 0
                        for h in (1, 0):
                            nc.tensor.matmul(
                                ps[h][:, j * ST + c0 : (j + 1) * ST],
                                lhsT=kT_sb[64 * h : 64 * h + 64,
                                           kb * KB : (kb + 1) * KB],
                                rhs=qT_sb[64 * h : 64 * h + 64,
                                          qt * ST + c0 : (qt + 1) * ST],
                                start=True,
                                stop=not diag,
                            )
                        if diag:
                            b0 = j * ST + 128 * rel
                            for h in (1, 0):
                                nc.tensor.matmul(
                                    ps[h][:, b0 : b0 + 128],
                                    lhsT=mask_sb[:, 0:KB],
                                    rhs=mask_sb[:, KB : 2 * KB],
                                    start=False,
                                    stop=True,
                                )
                    # exp: both heads into ONE [128, 2048] sbuf tile so the
                    # diagonal mask-multiplies cover both heads in one op
                    ext = exp_p.tile([128, 2, 2 * ST], bf16, tag="ex",
                                     name=f"ex{qt}_{pr}")
                    dve_h1 = False
                    if rels[1] < 0 and DVE_EXP_MOD:
                        full_cnt[0] += 1
                        dve_h1 = full_cnt[0] % DVE_EXP_MOD == 0
                    for h in (0, 1):
                        if rels[1] >= 0:  # diagonal pair: narrow exps that
                            # skip the never-read below-diagonal region
                            for j in range(2):
                                c0 = 128 * max(rels[j], 0)
                                nc.scalar.activation(
                                    ext[:, h, j * ST + c0 : (j + 1) * ST],
                                    ps[h][:, j * ST + c0 : (j + 1) * ST],
                                    EXP, scale=0.125,
                                )
                        elif h == 1 and dve_h1:
                            nc.vector.tensor_scalar(
                                ext[:, h, :].bitcast(mybir.dt.int16),
                                ps[h][:], EXP_K1, EXP_K2, MULT, ADD,
                            )
                        else:
                            nc.scalar.activation(ext[:, h, :], ps[h][:],
                                                 EXP, scale=0.125)
                    # attn@V lags one pair so PE never stalls on this exp
                    if prev is not None:
                        attnv(*prev)
                    prev = (pr, ext)
                    # interleave pending proj/Wo units, spread evenly
                    filler(-(npr - pr))
                attnv(*prev)
                if qt == NST - 1:
                    # tail: the serial normalize chain leaves the PE idle
                    # longer than the HAM window — keep it warm with dummy
                    # matmuls so the Wo tail runs at 2.4GHz, not 1.2
                    wu2 = psA.tile([128, ST], f32, tag="pp", name="warm2")
                    for i in range(14):
                        nc.tensor.matmul(
                            wu2[:, 0:KB],
                            lhsT=wq_sb[:, i % 4, :],
                            rhs=wq_sb[:, (i + 1) % 4, :],
                            start=True, stop=True,
                        )
                # normalize: woL[h*64:(h+1)*64, :] = po[h][64:128] / denom
                # (note: reciprocal_approx_fast requires base partition 0 —
                # nonzero-base operands return garbage on HW, probed)
                woL = wl_p.tile([128, ST], bf16, tag="wl", name=f"wl{qt}")
                for h in (0, 1):
                    r_sb = rsb_p.tile([DK, ST], f32, tag="r",
                                      name=f"r{qt}_{h}")
                    nc.vector.reciprocal_approx_fast(out=r_sb[:],
                                                     in_=po[h][0:64, :])
                    nc.vector.tensor_tensor(
                        out=woL[64 * h : 64 * h + 64, :],
                        in0=po[h][64:128, :], in1=r_sb[:], op=MULT,
                    )
                woL_tiles[qt] = woL

            def wo_units(qt):
                """8 closures, each one output block of the Wo projection.
                The two 512-col halves of a 128-row block share one sbuf
                tile and go to HBM as a single 1024-wide DMA."""
                wl = woL_tiles.pop(qt)
                obs = {}

                def unit(qb, nt):
                    def run():
                        q0 = qt * ST + qb * 128
                        pw = psA.tile([128, ST], f32, tag="pp",
                                      name=f"pw{qt}_{qb}_{nt}")
                        nc.tensor.matmul(
                            pw[:],
                            lhsT=wl[:, qb * 128 : (qb + 1) * 128],
                            rhs=woR[:, nt * ST : (nt + 1) * ST],
                            start=True, stop=True,
                        )
                        if nt == 0:
                            obs[qb] = oout_p.tile([128, 2 * ST], bf16,
                                                  tag="ob",
                                                  name=f"ob{qt}_{qb}")
                        ob = obs[qb]
                        if qt == NST - 1 and (qb + nt) % 2:
                            # tail: ACT is done with exps — share the
                            # PSUM eviction between Scalar and Vector
                            nc.scalar.copy(ob[:, nt * ST : (nt + 1) * ST],
                                           pw[:])
                        else:
                            nc.vector.tensor_copy(
                                ob[:, nt * ST : (nt + 1) * ST], pw[:])
                        if qt == NST - 1:
                            # tail: store each half as soon as it lands so
                            # the final DMA overlaps the remaining evicts
                            nc.sync.dma_start(
                                out=OUT[q0 : q0 + 128,
                                        nt * ST : (nt + 1) * ST],
                                in_=ob[:, nt * ST : (nt + 1) * ST],
                            )
                        elif nt == 1:
                            nc.sync.dma_start(
                                out=OUT[q0 : q0 + 128, :], in_=ob[:],
                            )
                    return run

                return [unit(qb, nt) for qb in range(4) for nt in range(2)]

            # two filler queues: projections gate the next tile's scores
            # (must drain at the tile boundary); Wo blocks are deferrable
            # and carry across boundaries so the boundary drain never
            # starves the ACT exp stream
            pend_proj = []
            pend_wo = []

            reserve = [4]

            def navail():
                return len(pend_proj) + len(pend_wo)

            def filler(n):
                # n < 0: spread -> emit ceil((len-R)/|n|), holding ~R units
                # back as PE filler for the next tile's ACT-bound start;
                # n > 0: emit up to n (force-drain)
                if n < 0:
                    n = -((navail() - reserve[0]) // n)
                for _ in range(min(n, navail())):
                    (pend_proj if pend_proj else pend_wo).pop(0)()

            for st in range(NST):
                if st == 0:
                    # q/k projections run inline (the first scores need
                    # them); the v projection overlaps the first exps as
                    # filler units
                    xq = get_in(0, "q", XQ, qin_p)
                    xk = get_in(0, "k", XK, kin_p)
                    xv = get_in(0, "v", XV, vin_p)
                    u0 = proj_units(0, xq, xk, xv)
                    for u in u0[:4]:
                        u()
                    pend_proj.extend(u0[4:])
                else:
                    # leftover proj units for this tile must precede its
                    # scores; deferred wo units carry over
                    while pend_proj:
                        pend_proj.pop(0)()
                if st + 1 < NST:
                    xq = get_in(st + 1, "q", XQ, qin_p)
                    xk = get_in(st + 1, "k", XK, kin_p)
                    xv = get_in(st + 1, "v", XV, vin_p)
                    pend_proj.extend(proj_units(st + 1, xq, xk, xv))
                if st >= 1:
                    # wo for qt=st-1: drained by filler inside attn(st)
                    pend_wo.extend(wo_units(st - 1))
                reserve[0] = 0 if st in (0, NST - 1) else 4
                attn(st, filler)
            # drain the tail
            filler(navail())
            pend_wo.extend(wo_units(NST - 1))
            filler(navail())

    nc.compile()
    return nc


def _prep_inputs(Q, K, V, Wq_w, Wq_b, Wk_w, Wk_b, Wv_w, Wv_b, Wo_w, Wo_b):
    bf = ml_dtypes.bfloat16
    f = np.float32

    def xprep(X):
        # [S, D] -> [128, 8, S]: x[p, t, s] = X[s, t*128+p]
        return np.ascontiguousarray(
            X[0].T.reshape(8, 128, S).transpose(1, 0, 2).astype(bf)
        )

    def wprep(Wslice):
        # Wslice [CD, D] -> [128, 8, CD]: w[p, t, c] = Wslice[c, t*128+p]
        return np.ascontiguousarray(
            Wslice.T.reshape(8, 128, CD).transpose(1, 0, 2).astype(bf)
        )

    XQp, XKp, XVp = xprep(Q), xprep(K), xprep(V)
    p = np.arange(KB)[:, None]
    fidx = np.arange(KB)[None, :]
    ident = np.eye(KB, dtype=np.float32)
    tri = np.where(p > fidx, -240.0, 0.0)
    msk = np.ascontiguousarray(np.concatenate([ident, tri], 1)).astype(bf)
    WoT = np.ascontiguousarray(Wo_w.T, dtype=f)  # [in, out]

    in_maps = []
    for c in range(NCORES):
        c0 = CD * c
        in_maps.append({
            "xq": XQp, "xk": XKp, "xv": XVp,
            "wq": wprep(Wq_w[c0 : c0 + CD, :]),
            "wk": wprep(Wk_w[c0 : c0 + CD, :]),
            "wv": wprep(Wv_w[c0 : c0 + CD, :]),
            "bq": np.ascontiguousarray(Wq_b[c0 : c0 + CD, None], dtype=f),
            "bk": np.ascontiguousarray(Wk_b[c0 : c0 + CD, None], dtype=f),
            "bvb": np.ascontiguousarray(
                np.broadcast_to(np.tile(Wv_b[c0 : c0 + CD], 4), (128, 512))
            ).astype(bf),
            "wor": np.ascontiguousarray(WoT[c0 : c0 + CD, :], dtype=bf),
            "msk": msk,
        })
    return in_maps


def _numpy_fallback(Q, K, V, Wq_w, Wq_b, Wk_w, Wk_b, Wv_w, Wv_b, Wo_w, Wo_b,
                    mask):
    q = (Q @ Wq_w.T + Wq_b).reshape(1, S, H, DK).transpose(0, 2, 1, 3)
    k = (K @ Wk_w.T + Wk_b).reshape(1, S, H, DK).transpose(0, 2, 1, 3)
    v = (V @ Wv_w.T + Wv_b).reshape(1, S, H, DK).transpose(0, 2, 1, 3)
    scores = np.einsum("bhqd,bhkd->bhqk", q, k) / np.sqrt(DK).astype(np.float32)
    scores = np.where(mask == 0, np.float32(-1e9), scores)
    scores -= scores.max(axis=-1, keepdims=True)
    e = np.exp(scores)
    attn = e / e.sum(axis=-1, keepdims=True)
    out = np.einsum("bhqk,bhkd->bhqd", attn, v)
    out = out.transpose(0, 2, 1, 3).reshape(1, S, D)
    return (out @ Wo_w.T + Wo_b).astype(np.float32)


def kernel(Q, K, V, Wq_w, Wq_b, Wk_w, Wk_b, Wv_w, Wv_b, Wo_w, Wo_b, mask,
           **run_kwargs):
    Q = np.asarray(Q); K = np.asarray(K); V = np.asarray(V)
    Wq_w = np.asarray(Wq_w); Wq_b = np.asarray(Wq_b)
    Wk_w = np.asarray(Wk_w); Wk_b = np.asarray(Wk_b)
    Wv_w = np.asarray(Wv_w); Wv_b = np.asarray(Wv_b)
    Wo_w = np.asarray(Wo_w); Wo_b = np.asarray(Wo_b)
    mask = np.asarray(mask)

    causal = np.array_equal(
        mask.reshape(S, S), np.tril(np.ones((S, S), mask.dtype))
    )
    if not causal:
        return _numpy_fallback(Q, K, V, Wq_w, Wq_b, Wk_w, Wk_b, Wv_w, Wv_b,
                               Wo_w, Wo_b, mask)

    from concourse.bass_utils import run_bass_kernel_spmd

    if _compiled[0] is None:
        _compiled[0] = _build()
    nc = _compiled[0]

    in_maps = _prep_inputs(Q, K, V, Wq_w, Wq_b, Wk_w, Wk_b, Wv_w, Wv_b,
                           Wo_w, Wo_b)
    res = run_bass_kernel_spmd(nc, in_maps, list(range(NCORES)), **run_kwargs)
    out = np.zeros((S, D), np.float32)
    for cres in res.results:
        out += np.asarray(cres["out"], dtype=np.float32)
    out += Wo_b.astype(np.float32)
    if run_kwargs:
        kernel.last_result = res
    return out.reshape(1, S, D).astype(np.float32)

